# revision 26
# baseline (speedup 1.0000x reference)
"""nn_CART_69355131895963 Trainium2 Bass kernel.

reference:
    BatchNorm1d(train-mode batch stats) -> per-tree sparsemax feature
    selection (einsum bf,tfs->tbs) -> sigmoid(xp - cut) -> per-tree
    [S,S] MLP layer + relu -> per-tree [S,O] layer -> mean over trees of
    o2 * tw.

Strategy (8 NeuronCores, batch-sharded 8192 rows/core; cost-model
facts that shaped it: matmul charges out-free-size x cycles/row with
fp8e4 DoubleRow at 0.5, ACT 0.83ns/elem/lane, DVE 1.04, GPSIMD is
SBUF-only and slow, DMA is one shared 360GB/s device):
  Host (O(params) only): sparsemax(fsm) -> P2 [F,TS] in feature-PAIR
    layout [128,2,TS] fp16; tw/T folded into W2; block-diagonal W1
    (4 trees/group); all small parameters laid out for SBUF use.
  Phase 1 (streamed over 8 x 1024-row chunks, DMA-bound ~30us):
    DMA x fp32 -> fp8e4 cast x8 (ACT/GPS split) and residual
    xr8 = fp8(x - x8) (GPS/DVE tensor_tensor) -> BN stats on PE as
    fp8 DoubleRow matmuls (cov-diag self-products + ones-sums, both
    b-subtile-paired) -> pair-transpose on PE (fp16 bitcast of fp8
    pairs x eye16, 8 b-tiles per [128,1024] f16 psum) -> ACT/DVE
    evicts giving xT16/xrT16 [128pairs, 8192] (= fp8 [f, b]).
  Phase 1.5: stats DRAM round-trip (AllReduce across cores; elided
    to write+read in the single-core sim program), half->pair layout
    conversion for free via the DMA APs; mean/var -> a=gamma*rsqrt;
    p2a8=fp8(a*P) + residual p2r8 (first m-slice emitted early so
    phase 2 unblocks); biasA = P^T(beta-mean*a) - cut via tiny
    matmuls; one dummy sigmoid hides the ACT table switch.
  Phase 2 (8 chunks of 1024 cols, A/B software pipeline with lag 2):
    A: xp = 3 accumulated fp8-DoubleRow matmuls per 512-col slice
       (p2a8*x8 + p2r8*x8 + p2a8*xr8: full F=256 contraction per
       instr, residual pair recovers ~fp16 accuracy at fp8 speed)
       -> ACT sigmoid(xp + biasA) -> score fp16.
    B: z2 = W1bd^T @ score (fp16 PE) -> relu+b1, DVE with every
       m==7 tile on ACT -> o1 fp16 [128ts, 8m, 1024].
    C (per 2 chunks, borrowing a z2 psum slot): out^T[128b,16] psum
       += o1(m)^T @ W2f(m) accumulated over 8 groups (16-col
       matmuls, 8x cheaper than the [16,b] orientation) -> DVE
       evict + bout bias -> DMA rows straight to OUT [8192, 16].
  Host: concat per-core outputs along batch. No host-side transpose.
  TimelineSim: 127.2us (baseline 208.3us); rel err 1.2e-3.
"""

import numpy as np

import concourse.tile as tile
from concourse import bacc, mybir
from concourse.bass_utils import run_bass_kernel_spmd

f8 = mybir.dt.float8e4
f16 = mybir.dt.float16
f32 = mybir.dt.float32
AF = mybir.ActivationFunctionType
ALU = mybir.AluOpType
DRM = mybir.MatmulPerfMode.DoubleRow

N_CORES = 8
B_TOTAL = 65536
BS = B_TOTAL // N_CORES     # 8192 rows per core
F = 256
T = 32
S = 32
O = 16
TS = T * S                  # 1024
NM = TS // 128              # 8 ts-tiles (tree groups of 4)
BN_EPS = 1e-5

ROWS1 = 1024                # phase-1 chunk rows
NCH1 = BS // ROWS1          # 8
SUB1 = ROWS1 // 128         # 8 b-subtiles per chunk

CH = 1024                   # phase-2 chunk columns
NCH = BS // CH              # 8


def _sparsemax_cols(z):
    """sparsemax along axis 0 of z [F, C] (float64)."""
    zs = np.sort(z, axis=0)[::-1]
    k = np.arange(1, z.shape[0] + 1)[:, None]
    cs = np.cumsum(zs, axis=0)
    support = (1.0 + k * zs) > cs
    ksup = support.sum(0)
    tau = (cs[ksup - 1, np.arange(z.shape[1])] - 1.0) / ksup
    return np.maximum(z - tau, 0.0)


def _host_prep(gamma, beta, fsm, cut, W1, b1, W2, b2, tw):
    import ml_dtypes
    P2 = _sparsemax_cols(
        fsm.astype(np.float64).transpose(1, 0, 2).reshape(F, TS)
    ).astype(np.float32)
    p2pr = np.ascontiguousarray(P2.reshape(128, 2, TS)).astype(np.float16)
    cutv = cut.reshape(TS).reshape(NM, 128).T.copy().astype(np.float32)
    b1v = b1.reshape(TS).reshape(NM, 128).T.copy().astype(np.float32)

    w1bd = np.zeros((NM, 128, 128), dtype=np.float32)
    for g in range(NM):
        for i in range(4):
            w1bd[g, 32 * i:32 * i + 32, 32 * i:32 * i + 32] = W1[4 * g + i]
    w1bd = w1bd.transpose(1, 0, 2).astype(np.float16).copy()

    w2f = (W2 * (tw / T)).reshape(TS, O).astype(np.float32) \
        .reshape(NM, 128, O).transpose(1, 0, 2).astype(np.float16).copy()
    bout = (b2 * (tw / T)).sum(0).reshape(O).astype(np.float32)
    boutbc = np.ascontiguousarray(
        np.broadcast_to(bout[None, None, :], (128, NM, O)), dtype=np.float32)

    gpair = gamma.reshape(128, 2).copy().astype(np.float32)
    bpair = beta.reshape(128, 2).copy().astype(np.float32)
    eye16 = np.eye(128, dtype=np.float16)
    ones8 = np.ones((128, 2, 1), dtype=ml_dtypes.float8_e4m3)
    return dict(p2pr=p2pr, cutv=cutv, b1v=b1v, w1bd=w1bd, w2f=w2f,
                boutbc=boutbc, gpair=gpair, bpair=bpair,
                eye16=eye16, ones8=ones8)


def build_program(repeat=1, single_core_sim=False, LAG=2, CDELAY=2,
                  ACTRELU=8, SCBUFS=3, S3G=2, O1BUFS=3):
    """Trace + compile the SPMD Bass program (identical on all 8 cores).

    single_core_sim=True builds the same per-core program with the
    cross-core AllReduce elided (for cost-model simulation only).
    """
    ncores = 1 if single_core_sim else N_CORES
    nc = bacc.Bacc("TRN2", target_bir_lowering=False, debug=False,
                   num_devices=ncores)
    X = nc.dram_tensor("x", [BS, F], f32, kind="ExternalInput").ap()
    P2PR = nc.dram_tensor("p2pr", [128, 2, TS], f16, kind="ExternalInput").ap()
    CUTV = nc.dram_tensor("cutv", [128, NM], f32, kind="ExternalInput").ap()
    B1V = nc.dram_tensor("b1v", [128, NM], f32, kind="ExternalInput").ap()
    W1BD = nc.dram_tensor("w1bd", [128, NM, 128], f16, kind="ExternalInput").ap()
    W2F = nc.dram_tensor("w2f", [128, NM, O], f16, kind="ExternalInput").ap()
    BOUTBC = nc.dram_tensor("boutbc", [128, NM, O], f32, kind="ExternalInput").ap()
    GPAIR = nc.dram_tensor("gpair", [128, 2], f32, kind="ExternalInput").ap()
    BPAIR = nc.dram_tensor("bpair", [128, 2], f32, kind="ExternalInput").ap()
    EYE16 = nc.dram_tensor("eye16", [128, 128], f16, kind="ExternalInput").ap()
    ONES8 = nc.dram_tensor("ones8", [128, 2, 1], f8, kind="ExternalInput").ap()
    OUT = nc.dram_tensor("out", [BS, O], f32, kind="ExternalOutput").ap()

    Xv = X.rearrange("(n p) f -> p n f", p=128)

    with tile.TileContext(nc) as tc:
        with tc.tile_pool(name="const", bufs=1) as pc, \
             tc.tile_pool(name="xt", bufs=1) as pxt, \
             tc.tile_pool(name="dram", bufs=1, space="DRAM") as pdram:

            def load_const(name, shape, dt, src):
                t = pc.tile(shape, dt, name=name)
                nc.sync.dma_start(t[:], src[:])
                return t

            # small consts needed early in phase 1
            eye16 = load_const("eye16_sb", [128, 128], f16, EYE16)
            ones8 = load_const("ones8_sb", [128, 2, 1], f8, ONES8)

            # big consts issued AFTER the x DMAs (loaded lazily below)
            big = {}

            def load_big_consts():
                big["p2pr"] = load_const("p2pr_sb", [128, 2, TS], f16, P2PR)
                big["cutv"] = load_const("cutv_sb", [128, NM], f32, CUTV)
                big["b1v"] = load_const("b1v_sb", [128, NM], f32, B1V)
                big["w1bd"] = load_const("w1bd_sb", [128, NM, 128], f16, W1BD)
                big["w2f"] = load_const("w2f_sb", [128, NM, O], f16, W2F)
                big["boutbc"] = load_const("boutbc_sb", [128, NM, O], f32,
                                           BOUTBC)
                big["gpair"] = load_const("gpair_sb", [128, 2], f32, GPAIR)
                big["bpair"] = load_const("bpair_sb", [128, 2], f32, BPAIR)

            # xT16[p, b] (fp16-typed) = fp8 pair (16*x[b, 2p], 16*x[b, 2p+1])
            xT16 = pxt.tile([128, BS], f16, name="xt16")
            xrT16 = pxt.tile([128, BS], f16, name="xrt16")

            def body_once():
                # ---------- phase 1: load, cast fp8, stats, transpose ----
                stat_half = pc.tile([128, 2, 2], f32, name="stat_half")
                with tc.tile_pool(name="ph1", bufs=2) as p1, \
                     tc.tile_pool(name="x32p", bufs=NCH1) as p1x, \
                     tc.tile_pool(name="ph1ps", bufs=1, space="PSUM") as pst, \
                     tc.tile_pool(name="trps", bufs=2, space="PSUM") as ptr:
                    # issue the whole x load stream first: DMA is the
                    # serial prefix floor, nothing may queue ahead of it
                    x32s = []
                    for c in range(NCH1):
                        x32 = p1x.tile([128, SUB1, F], f32, tag="x32",
                                       name="x32")
                        nc.sync.dma_start(x32[:],
                                          Xv[:, c * SUB1:(c + 1) * SUB1, :])
                        x32s.append(x32)
                    if not big:
                        load_big_consts()
                    covP = [pst.tile([128, 128], f32, tag=f"cov{i}",
                                     name=f"cov{i}") for i in range(2)]
                    sumP = [pst.tile([128, 1], f32, tag=f"sum{i}",
                                     name=f"sum{i}") for i in range(2)]
                    for c in range(NCH1):
                        x32 = x32s[c]
                        # x8 = fp8(16*x); xr8 = fp8(16*x - x8): residual
                        # pair recovers fp16-grade accuracy from fp8 matmuls
                        x8 = p1.tile([128, SUB1, F], f8, tag="x8", name="x8")
                        nc.scalar.copy(x8[:, 0:6, :], x32[:, 0:6, :])
                        nc.gpsimd.tensor_copy(x8[:, 6:8, :], x32[:, 6:8, :])
                        xr8 = p1.tile([128, SUB1, F], f8, tag="xr8",
                                      name="xr8")
                        nc.gpsimd.tensor_tensor(xr8[:, 0:3, :],
                                                x32[:, 0:3, :], x8[:, 0:3, :],
                                                op=ALU.subtract)
                        nc.vector.tensor_tensor(xr8[:, 3:8, :],
                                                x32[:, 3:8, :], x8[:, 3:8, :],
                                                op=ALU.subtract)
                        # stats: fp8 DoubleRow over b-subtile pairs
                        for a in range(SUB1 // 2):
                            first = (c == 0 and a == 0)
                            last = (c == NCH1 - 1 and a == SUB1 // 2 - 1)
                            sl = x8[:, 2 * a:2 * a + 2, :]
                            for i in range(2):
                                fs = sl[:, :, 128 * i:128 * (i + 1)]
                                nc.tensor.matmul(covP[i][:], fs, fs,
                                                 start=first, stop=last,
                                                 perf_mode=DRM,
                                                 skip_group_check=True)
                                nc.tensor.matmul(sumP[i][:], fs, ones8[:],
                                                 start=first, stop=last,
                                                 perf_mode=DRM,
                                                 skip_group_check=True)
                        # pair-transpose: 8 b-tiles per psum buf, then evict
                        for src_t, dst in ((x8, xT16), (xr8, xrT16)):
                            for g in range(SUB1 // 8):
                                ztr = ptr.tile([128, 1024], f16, tag="ztr",
                                               name="ztr")
                                for t in range(8):
                                    bt = 8 * g + t
                                    nc.tensor.matmul(
                                        ztr[:, 128 * t:128 * (t + 1)],
                                        src_t[:, bt, :].bitcast(f16),
                                        eye16[:],
                                        is_transpose=True, start=True,
                                        stop=True, skip_group_check=True)
                                col = c * ROWS1 + g * 1024
                                if (g % 2 == 0) == (src_t is x8):
                                    nc.vector.tensor_copy(
                                        dst[:, col:col + 1024], ztr[:])
                                else:
                                    nc.scalar.copy(dst[:, col:col + 1024],
                                                   ztr[:])
                    # gather stats: [128, 2(i), 2(kind)] in F-half layout
                    for i in range(2):
                        tmp = p1.tile([128, 128], f32, tag="dtmp", name="dtmp")
                        nc.vector.tensor_tensor(tmp[:], covP[i][:],
                                                eye16[:], op=ALU.mult)
                        nc.vector.reduce_sum(stat_half[:, i, 1:2], tmp[:],
                                             axis=mybir.AxisListType.X)
                        nc.vector.tensor_copy(stat_half[:, i, 0:1],
                                              sumP[i][:])

                # ---------- phase 1.5: all-reduce + BN fold ----------
                # CC buffer is f-major [F, 2] so each leg is ONE dma call
                ccin = pdram.tile([F, 2], f32, name="ccin")
                ccout = pdram.tile([F, 2], f32, name="ccout")
                # f = 128*i + p (F-half layout) on the way out
                nc.sync.dma_start(
                    ccin[:].rearrange("(i p) k -> p i k", p=128),
                    stat_half[:])
                if single_core_sim:
                    ccred = ccin   # collective elided: read partials back
                else:
                    nc.gpsimd.collective_compute(
                        "AllReduce", ALU.add,
                        replica_groups=[list(range(N_CORES))],
                        ins=[ccin.opt()], outs=[ccout.opt()])
                    ccred = ccout
                # read back in PAIR layout: f = 2p + j
                stat_pair = pc.tile([128, 2, 2], f32, name="stat_pair")
                nc.sync.dma_start(
                    stat_pair[:],
                    ccred[:].rearrange("(p j) k -> p j k", p=128))

                p2pr, cutv, b1v = big["p2pr"], big["cutv"], big["b1v"]
                w1bd, w2f, boutbc = big["w1bd"], big["w2f"], big["boutbc"]

                mom = pc.tile([128, 2, 2], f32, name="mom")
                nc.vector.tensor_scalar(mom[:], stat_pair[:], 1.0 / B_TOTAL,
                                        None, op0=ALU.mult)
                mean = mom[:, :, 0]
                ex2 = mom[:, :, 1]
                var = pc.tile([128, 2], f32, name="var")
                nc.vector.tensor_tensor(var[:], mean, mean, op=ALU.mult)
                nc.vector.tensor_tensor(var[:], ex2, var[:], op=ALU.subtract)
                eps = pc.tile([128, 1], f32, name="eps")
                nc.vector.memset(eps[:], BN_EPS)
                se = pc.tile([128, 2], f32, name="se")
                nc.scalar.activation(se[:], var[:], AF.Sqrt, bias=eps[:])
                sinv = pc.tile([128, 2], f32, name="sinv")
                nc.vector.reciprocal(sinv[:], se[:])
                av = pc.tile([128, 2], f32, name="av")
                nc.vector.tensor_tensor(av[:], sinv[:], big["gpair"][:],
                                        op=ALU.mult)
                cv = pc.tile([128, 2], f32, name="cv")
                nc.vector.tensor_tensor(cv[:], mean, av[:], op=ALU.mult)
                nc.vector.tensor_tensor(cv[:], big["bpair"][:], cv[:],
                                        op=ALU.subtract)
                cv16 = pc.tile([128, 2], f16, name="cv16")
                nc.vector.tensor_copy(cv16[:], cv[:])

                # p2a8 = fp8(a*P); p2r8 = fp8(a*P - p2a8): small m=0
                # slice first so stageA(0) unblocks early, then the bulk
                p2a8 = pc.tile([128, 2, TS], f8, name="p2a8")
                p2r8 = pc.tile([128, 2, TS], f8, name="p2r8")
                for ms in (slice(0, 128), slice(128, TS)):
                    nc.vector.tensor_scalar(p2a8[:, 0, ms], p2pr[:, 0, ms],
                                            av[:, 0:1], None, op0=ALU.mult)
                    nc.scalar.activation(p2a8[:, 1, ms], p2pr[:, 1, ms],
                                         AF.Copy, scale=av[:, 1:2])
                    nc.vector.scalar_tensor_tensor(
                        p2r8[:, 0, ms], p2pr[:, 0, ms], av[:, 0:1],
                        p2a8[:, 0, ms], op0=ALU.mult, op1=ALU.subtract)
                    nc.vector.scalar_tensor_tensor(
                        p2r8[:, 1, ms], p2pr[:, 1, ms], av[:, 1:2],
                        p2a8[:, 1, ms], op0=ALU.mult, op1=ALU.subtract)
                    if ms.start == 0:
                        # act-table switch load overlaps the remaining fold
                        dumm = pc.tile([128, 1], f16, name="dumm")
                        nc.scalar.activation(dumm[:], av[:, 0:1], AF.Sigmoid)
                biasA = pc.tile([128, NM], f32, name="biasA")
                with tc.tile_pool(name="dps", bufs=1, space="PSUM") as pdp:
                    dP = pdp.tile([128, NM], f32, name="dP")
                    for m in range(NM):
                        for j in range(2):
                            nc.tensor.matmul(
                                dP[:, m:m + 1],
                                p2pr[:, j, 128 * m:128 * (m + 1)],
                                cv16[:, j:j + 1],
                                start=(j == 0), stop=(j == 1),
                                skip_group_check=True)
                    nc.vector.tensor_tensor(biasA[:], dP[:], cutv[:],
                                            op=ALU.subtract)

                # fp8 views: [p, j, b] with j the feature-pair lane
                xT8 = xT16[:].bitcast(f8).rearrange("p (b j) -> p j b", j=2)
                xrT8 = xrT16[:].bitcast(f8).rearrange("p (b j) -> p j b", j=2)

                # ---------- phase 2: software-pipelined tree forest ------
                with tc.tile_pool(name="xpps", bufs=2, space="PSUM") as pxp, \
                     tc.tile_pool(name="z2ps", bufs=2, space="PSUM") as pz2, \
                     tc.tile_pool(name="sc", bufs=SCBUFS) as psc, \
                     tc.tile_pool(name="o1", bufs=O1BUFS) as po1, \
                     tc.tile_pool(name="osb", bufs=2) as pos:
                    NJ = NCH * NM
                    scs, o1cs = {}, {}

                    def stageA(j):
                        c, m = divmod(j, NM)
                        xp = pxp.tile([128, CH], f32, tag="xp", name="xp")
                        msl = slice(128 * m, 128 * (m + 1))
                        for q in range(CH // 512):
                            csl = slice(c * CH + 512 * q,
                                        c * CH + 512 * (q + 1))
                            terms = ((p2a8, xT8), (p2r8, xT8), (p2a8, xrT8))
                            for k, (lt, rt) in enumerate(terms):
                                nc.tensor.matmul(
                                    xp[:, 512 * q:512 * (q + 1)],
                                    lt[:, :, msl], rt[:, :, csl],
                                    start=(k == 0), stop=(k == 2),
                                    perf_mode=DRM, skip_group_check=True)
                        sc = psc.tile([128, CH], f16, tag="sc", name="sc")
                        nc.scalar.activation(sc[:], xp[:], AF.Sigmoid,
                                             bias=biasA[:, m:m + 1])
                        scs[j] = sc

                    def stageB(j):
                        c, m = divmod(j, NM)
                        sc = scs.pop(j)
                        if m == 0:
                            o1cs[c] = po1.tile([128, NM, CH], f16, tag="o1",
                                               name=f"o1c{c}")
                        z2 = pz2.tile([128, CH], f32, tag="z2", name="z2")
                        for q in range(CH // 512):
                            nc.tensor.matmul(z2[:, 512 * q:512 * (q + 1)],
                                             w1bd[:, m, :],
                                             sc[:, 512 * q:512 * (q + 1)],
                                             start=True, stop=True,
                                             skip_group_check=True)
                        # GPSIMD cannot read PSUM; split relu DVE/ACT
                        if ACTRELU and j % NM == ACTRELU - 1:
                            nc.scalar.activation(o1cs[c][:, m, :], z2[:],
                                                 AF.Relu,
                                                 bias=b1v[:, m:m + 1])
                        else:
                            nc.vector.tensor_scalar(o1cs[c][:, m, :], z2[:],
                                                    b1v[:, m:m + 1], 0.0,
                                                    op0=ALU.add, op1=ALU.max)

                    def stageC(cg):
                        # one psum borrow (z2-tag) covers S3G chunks
                        outT = pz2.tile([128, S3G, NM, O], f32, tag="z2",
                                        name=f"outT{cg}")
                        for ci in range(S3G):
                            c = cg * S3G + ci
                            o1c = o1cs.pop(c)
                            for bt in range(CH // 128):
                                for m in range(NM):
                                    nc.tensor.matmul(
                                        outT[:, ci, bt, :],
                                        o1c[:, m, 128 * bt:128 * (bt + 1)],
                                        w2f[:, m, :],
                                        start=(m == 0), stop=(m == NM - 1),
                                        skip_group_check=True)
                        osb = pos.tile([128, S3G, NM, O], f32, tag="osb",
                                       name="osb")
                        for ci in range(S3G):
                            nc.vector.tensor_tensor(osb[:, ci, :, :],
                                                    outT[:, ci, :, :],
                                                    boutbc[:], op=ALU.add)
                        cg0 = cg * S3G * CH
                        nc.sync.dma_start(
                            OUT[cg0:cg0 + S3G * CH, :]
                               .rearrange("(s p) o -> p s o", p=128),
                            osb[:].rearrange("p g s o -> p (g s) o"))

                    # lag stageB so its PE matmuls never park in the
                    # 4-deep wait queue and block s1 issue (ACT starvation)
                    for j in range(NJ + LAG + CDELAY):
                        if j < NJ:
                            stageA(j)
                        jb = j - LAG
                        if 0 <= jb < NJ:
                            stageB(jb)
                        jc = j - LAG - CDELAY
                        if jc >= 0 and jc % (NM * S3G) == NM * S3G - 1:
                            stageC(jc // (NM * S3G))

            for _rep in range(repeat):
                body_once()
    nc.compile()
    return nc


_NC_CACHE = {}


def _get_program(repeat=1):
    if repeat not in _NC_CACHE:
        _NC_CACHE[repeat] = build_program(repeat)
    return _NC_CACHE[repeat]


def make_in_maps(inputs):
    x = np.ascontiguousarray(inputs["x"], dtype=np.float32)
    params = _host_prep(np.asarray(inputs["gamma"]), np.asarray(inputs["beta"]),
                        np.asarray(inputs["fsm"]), np.asarray(inputs["cut"]),
                        np.asarray(inputs["W1"]), np.asarray(inputs["b1"]),
                        np.asarray(inputs["W2"]), np.asarray(inputs["b2"]),
                        np.asarray(inputs["tw"]))
    return [{"x": x[c * BS:(c + 1) * BS], **params} for c in range(N_CORES)]


def kernel(x, gamma, beta, fsm, cut, W1, b1, W2, b2, tw):
    """Full unsharded inputs in, full [B, O] float32 output out."""
    inputs = dict(x=x, gamma=gamma, beta=beta, fsm=fsm, cut=cut, W1=W1,
                  b1=b1, W2=W2, b2=b2, tw=tw)
    nc = _get_program(repeat=1)
    in_maps = make_in_maps(inputs)
    res = run_bass_kernel_spmd(nc, in_maps, core_ids=list(range(N_CORES)))
    out = np.concatenate([res.results[c]["out"] for c in range(N_CORES)],
                         axis=0)
    return np.ascontiguousarray(out, dtype=np.float32)
